# revision 11
# baseline (speedup 1.0000x reference)
"""CAMSA multi-mask attention kernel for one TRN2 chip (8 NeuronCores).

Problem: B=4, S=2048, D=1024, M=4 stride masks.
  Q = x@Wq + bq ; K = x@Wk + bk ; V = x@Wv + bv
  scores = Q K^T / sqrt(D)                    [B,S,S]
  weights_m = softmax(where(mask_m==0, -1e9, scores))   m = 0..3
  out = (mean_m weights_m) @ V @ Wo + bo

Key algebra: with P = exp(scores) (no row-max needed; scores ~ N(0,1)),
  weights_m = (mask_m * P) / den_m,   den_m[q] = sum_k mask_m[q,k] P[q,k]
  Wsum = sum_m inv_m * (mask_m * P),  inv_m = 1/(M * den_m)
  out = Wsum @ V @ Wo
One P, per-mask denominators via fused tensor_tensor_reduce, one WV matmul.

Sharding: core c = (batch b=c//2, query-half h=c%2): 1024 query rows each,
full 2048 keys. K/V projections are computed redundantly by the two cores
sharing a batch (no collectives needed).

Layouts on device (all matmul operands bf16, contraction dim on partitions):
  xT   [dx, r]   (host-pre-transposed)  -> operands for projections
  QT   [dout, q]  KT [dout, k]  (projection outputs, transposed layout)
  V    [k, dout]
  scores[q-tile, k] = QT_tile^T . KT            (PSUM, then ACT exp -> P bf16)
  masks int32 -> bf16 via gpsimd cast-DMA; denominators via tensor_tensor_reduce
  Wsum [q, k] -> WsumT [k, q] via HWDGE xbar DMA transpose
  out_preT[d, q] = V_tile^T . WsumT ; final[q, dout] = out_preT_tile^T . Wo
"""

import numpy as np

B, S, D, M = 4, 2048, 1024, 4
SQ = S // 2          # query rows per core
PART = 128
N_CORES = 8

_CACHE = {}


def build(nc_factory=None, S=S, D=D, SQ=SQ, M=M, chunked_transpose=False, stop_phase=99):
    from concourse import bass, mybir, bacc, tile

    fp32 = mybir.dt.float32
    bf16 = mybir.dt.bfloat16
    i32 = mybir.dt.int32
    AF = mybir.ActivationFunctionType
    ALU = mybir.AluOpType

    P = PART
    DCH = D // P         # d-chunks
    KCH = S // P         # key-row chunks
    QTILES = SQ // P     # q-tiles per core
    NB = min(512, S, SQ, D)     # moving free-dim block

    if nc_factory is None:
        nc = bacc.Bacc("TRN2", target_bir_lowering=False, debug=False,
                       num_devices=N_CORES)
    else:
        nc = nc_factory()

    xT_d = nc.dram_tensor("xT", [D, S], fp32, kind="ExternalInput")
    xTq_d = nc.dram_tensor("xTq", [D, SQ], fp32, kind="ExternalInput")
    mk_d = nc.dram_tensor("mk", [M, SQ, S], i32, kind="ExternalInput")
    wq_d = nc.dram_tensor("Wq", [D, D], fp32, kind="ExternalInput")
    wk_d = nc.dram_tensor("Wk", [D, D], fp32, kind="ExternalInput")
    wv_d = nc.dram_tensor("Wv", [D, D], fp32, kind="ExternalInput")
    wo_d = nc.dram_tensor("Wo", [D, D], fp32, kind="ExternalInput")
    out_d = nc.dram_tensor("out", [SQ, D], fp32, kind="ExternalOutput")

    with tile.TileContext(nc) as tc:
        with tc.tile_pool(name="persist", bufs=1) as pp, \
             tc.tile_pool(name="psum", bufs=6, space="PSUM") as psp:

            # ---- persistent SBUF tensors -------------------------------
            QT = pp.tile([P, DCH * SQ], bf16)    # [p, j*SQ + q] = Q[q, j*128+p]
            KT = pp.tile([P, DCH * S], bf16)     # [p, j*S + k]  = K[k, j*128+p]
            V = pp.tile([P, KCH * D], bf16)      # [p, i*D + d]  = V[i*128+p, d]
            WT = pp.tile([P, KCH * SQ], bf16)    # [p, i*SQ + q] = Wsum[q, i*128+p]
            OT = QT                              # QT dead by phase F; reuse
            #     [p, j*SQ + q] = out_pre[q, j*128+p]
            if stop_phase < 99:   # truncated bisect builds: keep scheduler happy
                for _tile in (QT, KT, V, WT):
                    nc.gpsimd.memset(_tile[:], 0.0)

            def proj(dst, w_sb, src_sb, ncols):
                # dst[p, j*ncols + r] = sum_dx W[dx, j*128+p] * src[dx, r]
                for j in range(DCH):
                    for qb in range(ncols // NB):
                        ps = psp.tile([P, NB], fp32, tag="ps")
                        for c in range(DCH):
                            nc.tensor.matmul(
                                ps[:],
                                w_sb[:, c * D + j * P: c * D + (j + 1) * P],
                                src_sb[:, c * ncols + qb * NB: c * ncols + (qb + 1) * NB],
                                start=(c == 0), stop=(c == DCH - 1))
                        nc.any.tensor_copy(
                            dst[:, j * ncols + qb * NB: j * ncols + (qb + 1) * NB],
                            ps[:])

            # ---- phase A/B: load + projections -------------------------
            with tc.tile_pool(name="stage_x", bufs=1) as sx:
                xT = sx.tile([P, DCH * S], bf16)
                xTq = sx.tile([P, DCH * SQ], bf16)
                nc.gpsimd.dma_start(
                    xTq[:].rearrange("p (c r) -> p c r", c=DCH),
                    xTq_d.ap().rearrange("(c p) r -> p c r", p=P))
                nc.gpsimd.dma_start(
                    xT[:].rearrange("p (c r) -> p c r", c=DCH),
                    xT_d.ap().rearrange("(c p) r -> p c r", p=P))

                with tc.tile_pool(name="stage_w", bufs=1) as sw:
                    Wq = sw.tile([P, DCH * D], bf16, tag="wtile")
                    nc.gpsimd.dma_start(
                        Wq[:].rearrange("p (c d) -> p c d", c=DCH),
                        wq_d.ap().rearrange("(c p) d -> p c d", p=P))
                    if stop_phase >= 2:
                        proj(QT, Wq, xTq, SQ)
                with tc.tile_pool(name="stage_w2", bufs=1) as sw:
                    Wk = sw.tile([P, DCH * D], bf16, tag="wtile")
                    nc.gpsimd.dma_start(
                        Wk[:].rearrange("p (c d) -> p c d", c=DCH),
                        wk_d.ap().rearrange("(c p) d -> p c d", p=P))
                    if stop_phase >= 2:
                        proj(KT, Wk, xT, S)
                with tc.tile_pool(name="stage_w3", bufs=1) as sw:
                    Wv = sw.tile([P, DCH * D], bf16, tag="wtile")
                    nc.gpsimd.dma_start(
                        Wv[:].rearrange("p (c d) -> p c d", c=DCH),
                        wv_d.ap().rearrange("(c p) d -> p c d", p=P))
                    # V[r, dout]: lhsT = xT chunk tile, rhs = Wv
                    for i in range(KCH if stop_phase >= 2 else 0):
                        for db in range(D // NB):
                            ps = psp.tile([P, NB], fp32, tag="ps")
                            for c in range(DCH):
                                nc.tensor.matmul(
                                    ps[:],
                                    xT[:, c * S + i * P: c * S + (i + 1) * P],
                                    Wv[:, c * D + db * NB: c * D + (db + 1) * NB],
                                    start=(c == 0), stop=(c == DCH - 1))
                            nc.any.tensor_copy(
                                V[:, i * D + db * NB: i * D + (db + 1) * NB],
                                ps[:])

            # ---- phase C/D/E: scores -> P -> mask softmax -> WsumT -----
            wk_ctx = tc.tile_pool(name="work", bufs=2)
            wkp = wk_ctx.__enter__()
            inv_scale = 1.0 / float(np.sqrt(np.float32(D)))
            for t in range(QTILES if stop_phase >= 3 else 0):
                Pt = wkp.tile([P, S], bf16, tag="Pt")
                for kb in range(S // NB):
                    ps = psp.tile([P, NB], fp32, tag="ps")
                    for c in range(DCH):
                        nc.tensor.matmul(
                            ps[:],
                            QT[:, c * SQ + t * P: c * SQ + (t + 1) * P],
                            KT[:, c * S + kb * NB: c * S + (kb + 1) * NB],
                            start=(c == 0), stop=(c == DCH - 1))
                    nc.scalar.activation(
                        Pt[:, kb * NB:(kb + 1) * NB], ps[:],
                        AF.Exp, scale=inv_scale)

                if stop_phase < 4:
                    continue
                mt = wkp.tile([P, M * S], bf16, tag="mt")
                nc.gpsimd.dma_start(
                    mt[:].rearrange("p (m k) -> p m k", m=M),
                    mk_d.ap()[:, t * P:(t + 1) * P, :].transpose([1, 0, 2]))

                if stop_phase == 31:
                    continue
                den = wkp.tile([P, M], fp32, tag="den")
                scr = wkp.tile([P, S], bf16, tag="scr")
                for m in range(M if stop_phase != 32 else 1):
                    # in-place: mt_m <- mt_m * Pt ; den_m = rowsum via ACT accum
                    # (tensor_tensor_reduce is broken on this HW/ucode)
                    nc.vector.tensor_tensor(
                        mt[:, m * S:(m + 1) * S],
                        mt[:, m * S:(m + 1) * S],
                        Pt[:], op=ALU.mult)
                    nc.scalar.activation(
                        scr[:], mt[:, m * S:(m + 1) * S],
                        AF.Copy, accum_out=den[:, m:m + 1])
                if stop_phase <= 33:
                    continue
                inv = wkp.tile([P, M], fp32, tag="inv")
                nc.vector.reciprocal(inv[:], den[:])
                # fold the 1/M mask-mean into inv
                nc.vector.tensor_scalar_mul(inv[:], inv[:], 1.0 / M)

                Wsum = wkp.tile([P, S], bf16, tag="Wsum")
                if stop_phase == 34:
                    continue
                nc.vector.tensor_scalar(
                    Wsum[:], mt[:, 0:S], inv[:, 0:1], None, op0=ALU.mult)
                for m in range(1, M):
                    nc.vector.scalar_tensor_tensor(
                        out=Wsum[:], in0=mt[:, m * S:(m + 1) * S],
                        scalar=inv[:, m:m + 1], in1=Wsum[:],
                        op0=ALU.mult, op1=ALU.add)

                # transpose Wsum [128, S] -> WT columns via xbar DMA
                if stop_phase < 5:
                    continue
                if chunked_transpose:
                    for i in range(KCH):
                        nc.sync.dma_start(
                            WT[:, i * SQ + t * P: i * SQ + (t + 1) * P],
                            Wsum[:, i * P:(i + 1) * P],
                            transpose=True)
                else:
                    nc.sync.dma_start_transpose(
                        WT[:].rearrange("p (i q) -> p i q", i=KCH)[:, :, t * P:(t + 1) * P],
                        Wsum[:])

            # ---- phase F: out_preT[d, q] = V^T-tiles . WsumT -----------
            for j in range(DCH if stop_phase >= 6 else 0):
                for qb in range(SQ // NB):
                    ps = psp.tile([P, NB], fp32, tag="ps")
                    for i in range(KCH):
                        nc.tensor.matmul(
                            ps[:],
                            V[:, i * D + j * P: i * D + (j + 1) * P],
                            WT[:, i * SQ + qb * NB: i * SQ + (qb + 1) * NB],
                            start=(i == 0), stop=(i == KCH - 1))
                    nc.any.tensor_copy(
                        OT[:, j * SQ + qb * NB: j * SQ + (qb + 1) * NB],
                        ps[:])

            # ---- phase G: final[q, dout] = out_preT-tiles^T . Wo -------
            with tc.tile_pool(name="stage_wo", bufs=1) as sw:
                Wo = sw.tile([P, DCH * D], bf16)
                nc.gpsimd.dma_start(
                    Wo[:].rearrange("p (c d) -> p c d", c=DCH),
                    wo_d.ap().rearrange("(c p) d -> p c d", p=P))
                for t in range(QTILES if stop_phase >= 7 else 0):
                    ot = wkp.tile([P, D], fp32, tag="ot")
                    for db in range(D // NB):
                        ps = psp.tile([P, NB], fp32, tag="ps")
                        for c in range(DCH):
                            nc.tensor.matmul(
                                ps[:],
                                OT[:, c * SQ + t * P: c * SQ + (t + 1) * P],
                                Wo[:, c * D + db * NB: c * D + (db + 1) * NB],
                                start=(c == 0), stop=(c == DCH - 1))
                        nc.any.tensor_copy(ot[:, db * NB:(db + 1) * NB], ps[:])
                    nc.sync.dma_start(out_d.ap()[t * P:(t + 1) * P, :], ot[:])
            wk_ctx.__exit__(None, None, None)

    nc.compile()
    return nc


def _get_nc():
    if "nc" not in _CACHE:
        _CACHE["nc"] = build()
    return _CACHE["nc"]


def kernel(x, stride_masks, Wq, bq, Wk, bk, Wv, bv, Wo, bo):
    from concourse import bass_utils

    x = np.ascontiguousarray(np.asarray(x, dtype=np.float32))
    stride_masks = np.ascontiguousarray(np.asarray(stride_masks, dtype=np.int32))
    Wq = np.asarray(Wq, dtype=np.float32)
    Wk = np.asarray(Wk, dtype=np.float32)
    Wv = np.asarray(Wv, dtype=np.float32)
    Wo = np.asarray(Wo, dtype=np.float32)
    bq = np.asarray(bq, dtype=np.float32)
    bk = np.asarray(bk, dtype=np.float32)
    bv = np.asarray(bv, dtype=np.float32)
    bo = np.asarray(bo, dtype=np.float32)

    nc = _get_nc()

    # biases in this problem are spec'd zero-fill; the device kernel omits
    # them. bv/bo fold in exactly on the host (softmax rows sum to 1);
    # bq/bk would need a device path, so assert they are zero.
    assert not (np.any(bq) or np.any(bk)), "nonzero q/k bias unsupported"

    mk_half = [np.ascontiguousarray(stride_masks[:, h * SQ:(h + 1) * SQ, :])
               for h in range(2)]
    in_maps = []
    for c in range(N_CORES):
        b, h = c // 2, c % 2
        xT = np.ascontiguousarray(x[b].T)
        xTq = np.ascontiguousarray(xT[:, h * SQ:(h + 1) * SQ])
        in_maps.append({
            "xT": xT, "xTq": xTq, "mk": mk_half[h],
            "Wq": Wq, "Wk": Wk, "Wv": Wv, "Wo": Wo,
        })

    res = bass_utils.run_bass_kernel_spmd(nc, in_maps, core_ids=list(range(N_CORES)))
    _CACHE["last_results"] = res

    out = np.empty((B, S, D), dtype=np.float32)
    for c in range(N_CORES):
        b, h = c // 2, c % 2
        out[b, h * SQ:(h + 1) * SQ, :] = res.results[c]["out"]

    if np.any(bv):
        out += (bv @ Wo)[None, None, :]
    if np.any(bo):
        out += bo[None, None, :]
    return out
